# revision 36
# baseline (speedup 1.0000x reference)
"""Trainium2 Bass kernel for nn_Attention_58428735095559.

Paged-KV-cache GQA causal prefill attention:
  B=8 seqs x S=1024 tokens, 32 q-heads / 8 kv-heads, head_dim=128.
  reference: scatter k/v into a 16384-slot cache by slot_mapping, gather
  per-token KV by seq_slot_mapping, then causal GQA attention.

Sharding: tensor-parallel over heads across 8 cores. Core c owns kv-head c
and q-heads 4c..4c+3.

Host prep (not on the device critical path):
  - resolve scatter->gather exactly (last write wins) and gather K_eff/V_eff
  - pre-transpose q and K_eff into [d, token] layout, append a ones column
    to V_eff (fused softmax-denominator trick), cast everything to bf16

Device kernel per core (bf16 matmuls, fp32 PSUM):
  - QK: sc[kv, q] = kT_chunk.T @ qT  (contraction over d on partitions),
    block-causal skip, packed into [128, 1024] PSUM chunks
  - exp on ACT in large chunk instructions (scale folded in), out bf16 SBUF
  - diagonal causal mask as a multiplicative upper-tri mask on DVE (4x bf16)
  - PV: ot[q, d+1] = ep_chunk.T @ [V|1]  accumulated over kv tiles in PSUM;
    column d holds the softmax denominator for free
  - DVE copies ot -> SBUF staging; DMA unnormalized output + denominators
  - final divide + relayout on host
"""

import numpy as np

try:
    import concourse.bass as bass  # noqa: F401
except ImportError:  # fresh shells without the repo on PYTHONPATH
    import sys

    for p in ("/opt/trn_rl_repo", "/root/.axon_site/_ro/trn_rl_repo"):
        if p not in sys.path:
            sys.path.insert(0, p)

import concourse.bass as bass  # noqa: F401
import concourse.bacc as bacc
import concourse.mybir as mybir
import concourse.tile as tile
from concourse.bass_utils import run_bass_kernel_spmd

# problem constants (hardcoded; kernel.py must be self-contained)
B, S = 8, 1024
NUM_HEADS, HEAD_DIM, NUM_KV_HEADS = 32, 128, 8
T = B * S
NUM_SLOTS = 16384
SCALE = 1.0 / float(np.sqrt(HEAD_DIM))
NCORES = 8
HPC = NUM_HEADS // NCORES  # q heads per core = 4
D = HEAD_DIM
P = 128
KT = S // P  # kv tiles per seq = 8
CHUNK = 1536  # score columns per PSUM chunk (3 banks)
TOTCOL = sum(S - P * ki for ki in range(KT))  # 4608 block-causal score cols

F32 = mybir.dt.float32
BF16 = mybir.dt.bfloat16
AF = mybir.ActivationFunctionType
ALU = mybir.AluOpType

# global column offset where kv-tile ki's q-range begins
GS = [0] * KT
for _ki in range(1, KT):
    GS[_ki] = GS[_ki - 1] + (S - P * (_ki - 1))


def _qk_chunks():
    """Pack the block-causal (ki, q-range) score pieces into CHUNK-column
    PSUM chunks, splitting at 512 (PSUM bank) and CHUNK boundaries.
    Returns [ [(off_in_chunk, length, ki, qlo), ...] per chunk ]."""
    nchunk = (TOTCOL + CHUNK - 1) // CHUNK
    chunks = [[] for _ in range(nchunk)]
    g = 0
    for ki in range(KT):
        qcur = P * ki
        rem = S - qcur
        while rem > 0:
            ci, off = divmod(g, CHUNK)
            ln = min(rem, 512 - (off % 512), CHUNK - off)
            chunks[ci].append((off, ln, ki, qcur))
            g += ln
            qcur += ln
            rem -= ln
    assert g == TOTCOL
    return chunks


QK_CHUNKS = _qk_chunks()
NCHUNK = len(QK_CHUNKS)


def build_model():
    nc = bacc.Bacc("TRN2", target_bir_lowering=False, debug=False)

    qT_t = nc.dram_tensor("qT", [HPC, P, T], BF16, kind="ExternalInput")
    kT_t = nc.dram_tensor("kT", [P, T], BF16, kind="ExternalInput")
    v1_t = nc.dram_tensor("v1", [P, T // P, D + 1], BF16, kind="ExternalInput")
    o_t = nc.dram_tensor("o", [HPC, B, P, KT, D + 1], BF16, kind="ExternalOutput")

    with tile.TileContext(nc) as tc:
        with (
            tc.tile_pool(name="constp", bufs=1) as constp,
            tc.tile_pool(name="epp", bufs=2) as epp,
            tc.tile_pool(name="stgp", bufs=6) as stgp,
            tc.tile_pool(name="scp", bufs=2, space="PSUM") as scp,
            tc.tile_pool(name="otp", bufs=2, space="PSUM") as otp,
        ):
            # resident inputs, in consumption order (h outer, s inner).
            # The first ~3 seqs' slices go on the ACT HWDGE ring (idle at
            # t=0); the bulk is issued from GPSIMD (SWDGE, separate queue)
            # interleaved into the emission loop so transfers stream in
            # behind the compute. Outputs own the SP HWDGE ring.
            # All inputs stream through the ACT HWDGE ring (idle at t=0) in
            # strict consumption order — the ring is FIFO, so the first
            # iterations' slices complete before the bulk regardless of how
            # the scheduler orders the issue instructions. Outputs own the
            # SP ring (packet-granular round-robin would otherwise starve
            # small-descriptor transfers behind big ones).
            qts = constp.tile([P, HPC * T], BF16, tag="qts", name="qts")
            kts = constp.tile([P, T], BF16, tag="kts")
            v1 = constp.tile([P, T // P, D + 1], BF16, tag="v1")
            # all inputs on the ACT HWDGE ring in consumption order (ring is
            # FIFO; packet-granular round-robin across rings would starve
            # small descriptors behind big ones). Outputs own the SP ring.
            S3 = 3 * S
            nc.scalar.dma_start(kts[:, 0:S], kT_t.ap()[:, 0:S])
            nc.scalar.dma_start(qts[:, 0:S], qT_t.ap()[0, :, 0:S])
            nc.scalar.dma_start(kts[:, S:S3], kT_t.ap()[:, S:S3])
            nc.scalar.dma_start(qts[:, S:S3], qT_t.ap()[0, :, S:S3])
            nc.scalar.dma_start(v1[:, 0 : 3 * KT, :], v1_t.ap()[:, 0 : 3 * KT, :])
            # late tiers on the SP ring: same 2KB descriptor size as the ACT
            # ring's early tiers (fair packet round-robin), big-descriptor
            # qT1-3 last in this ring's FIFO so they cannot starve anything.
            # Output DMAs queue behind them — stgp bufs=6 absorbs the delay.
            nc.sync.dma_start(kts[:, S3:T], kT_t.ap()[:, S3:T])
            nc.sync.dma_start(qts[:, S3:T], qT_t.ap()[0, :, S3:T])
            nc.sync.dma_start(v1[:, 3 * KT :, :], v1_t.ap()[:, 3 * KT :, :])
            for hh in range(1, HPC):
                nc.sync.dma_start(
                    qts[:, hh * T : (hh + 1) * T], qT_t.ap()[hh, :, :]
                )

            def emit_qk_chunk(h, s, ep, ci, split=None):
                """QK matmuls + exp + diag masks for one score chunk.
                split: optional column to break the exp into two ACTIVATEs
                (used on the last iteration so trailing PV can start early)."""
                pieces = QK_CHUNKS[ci]
                base = sum(
                    sum(ln for _, ln, _, _ in QK_CHUNKS[j]) for j in range(ci)
                )
                clen = sum(ln for _, ln, _, _ in pieces)
                sc = scp.tile([P, CHUNK], F32, tag="sc", name="sc")
                for off, ln, ki, qlo in pieces:
                    q0 = h * T + s * S + qlo
                    nc.tensor.matmul(
                        sc[:, off : off + ln],
                        lhsT=kts[:, s * S + P * ki : s * S + P * (ki + 1)],
                        rhs=qts[:, q0 : q0 + ln],
                        start=True,
                        stop=True,
                    )
                for lo, hi in (
                    [(0, clen)] if split is None else [(0, split), (split, clen)]
                ):
                    nc.scalar.activation(
                        ep[:, base + lo : base + hi],
                        sc[:, lo:hi],
                        AF.Exp,
                        scale=SCALE,
                    )
                # diag-block causal masks inside this chunk (GPSIMD; in-place)
                for ki in range(KT):
                    if base <= GS[ki] < base + clen:
                        dsl = ep[:, GS[ki] : GS[ki] + P]
                        # keep where q >= kv: iota = f - p >= 0
                        nc.gpsimd.affine_select(
                            out=dsl,
                            in_=dsl,
                            compare_op=ALU.is_ge,
                            fill=0.0,
                            base=0,
                            channel_multiplier=-1,
                            pattern=[[1, P]],
                        )

            def emit_pv(h, s, ep, stage, qb_range):
                """PV accumulation + staging copies for a range of q-blocks."""
                for qb in qb_range:
                    ot = otp.tile([P, D + 1], F32, tag="ot", name="ot")
                    for ki in range(qb + 1):
                        col = GS[ki] + P * (qb - ki)
                        nc.tensor.matmul(
                            ot[:, :],
                            lhsT=ep[:, col : col + P],
                            rhs=v1[:, KT * s + ki, :],
                            start=(ki == 0),
                            stop=(ki == qb),
                        )
                    nc.vector.tensor_copy(stage[:, qb, :], ot[:, :])

            # software-pipelined emission, h outer so inputs stream in order:
            # [QK(i) c0, c1] [PV(i-1) qb0-4] [QK(i) c2] [PV(i-1) qb5-7].
            # c2's matmuls must sit EARLY in the PE stream: the exp's wait
            # semaphore counts all preceding PE completions, so PV matmuls
            # emitted before c2 would delay chunk-2's exp by their runtime.
            # The first PV batch still covers the PE bubble while chunk 0's
            # exp vacates c2's PSUM slot. The last iteration interleaves its
            # own PV behind each chunk so the pipeline drains promptly.
            NIT = B * HPC
            prev = None
            for it in range(NIT + 1):
                cur = None
                if it < NIT:
                    h, s = divmod(it, B)
                    ep = epp.tile([P, TOTCOL], BF16, tag="ep", name="ep")
                    stage = stgp.tile(
                        [P, KT, D + 1], BF16, tag="stage", name="stage"
                    )
                    cur = (h, s, ep, stage)
                    for ci in range(NCHUNK - 1):
                        emit_qk_chunk(h, s, ep, ci)
                if prev is not None:
                    ph, ps, pep, pstage = prev
                    emit_pv(ph, ps, pep, pstage, range(0, 5))
                if cur is not None:
                    h, s, ep, stage = cur
                    if it == NIT - 1:
                        # split the last exp at chunk offset 1024 (global col
                        # 4096): qb4-5 read cols <= 4096, so they unblock
                        # after the first sub-ACTIVATE; only qb6-7 trail the
                        # very last one.
                        emit_qk_chunk(h, s, ep, NCHUNK - 1, split=1024)
                        if prev is not None:
                            ph, ps, pep, pstage = prev
                            emit_pv(ph, ps, pep, pstage, range(5, KT))
                            nc.sync.dma_start(
                                o_t.ap()[ph, ps, :, :, :], pstage[:, :, :]
                            )
                            prev = None
                        # tail: drain PV in dependency order, streaming the
                        # output DMA per q-block pair as copies complete
                        for qb in range(0, KT, 2):
                            emit_pv(h, s, ep, stage, range(qb, qb + 2))
                            nc.sync.dma_start(
                                o_t.ap()[h, s, :, qb : qb + 2, :],
                                stage[:, qb : qb + 2, :],
                            )
                        cur = None
                    else:
                        emit_qk_chunk(h, s, ep, NCHUNK - 1)
                if prev is not None:
                    ph, ps, pep, pstage = prev
                    emit_pv(ph, ps, pep, pstage, range(5, KT))
                    nc.sync.dma_start(o_t.ap()[ph, ps, :, :, :], pstage[:, :, :])
                prev = cur

    nc.compile()
    return nc


_NC = None


def _get_model():
    global _NC
    if _NC is None:
        _NC = build_model()
    return _NC


def _host_prep(q, k, v, k_cache, v_cache, slot_mapping, seq_slot_mapping):
    """Resolve the cache scatter/gather on host and build per-core inputs."""
    import ml_dtypes

    bf16 = ml_dtypes.bfloat16
    q = np.asarray(q, dtype=np.float32)
    k = np.asarray(k, dtype=np.float32)
    v = np.asarray(v, dtype=np.float32)
    k_cache = np.asarray(k_cache, dtype=np.float32)
    v_cache = np.asarray(v_cache, dtype=np.float32)
    sm = np.asarray(slot_mapping, dtype=np.int64)
    ssm = np.asarray(seq_slot_mapping, dtype=np.int64)

    # exact scatter->gather resolution (last write wins, like jax .at[].set)
    last_writer = np.full(NUM_SLOTS, -1, dtype=np.int64)
    last_writer[sm] = np.arange(T, dtype=np.int64)
    lw = last_writer[ssm]
    hit = lw >= 0
    lw_safe = np.where(hit, lw, 0)
    keff = np.where(hit[:, None], k[lw_safe], k_cache[ssm])  # [T, DKV] f32
    veff = np.where(hit[:, None], v[lw_safe], v_cache[ssm])

    qT_all = np.ascontiguousarray(q.T.astype(bf16))  # [4096, T]
    kT_all = np.ascontiguousarray(keff.T.astype(bf16))  # [1024, T]
    v1_all = np.empty((T, NUM_KV_HEADS, D + 1), dtype=bf16)
    v1_all[:, :, :D] = veff.reshape(T, NUM_KV_HEADS, D).astype(bf16)
    v1_all[:, :, D] = np.float32(1.0)

    in_maps = []
    for c in range(NCORES):
        qT = qT_all[c * HPC * D : (c + 1) * HPC * D].reshape(HPC, P, T)
        kT = kT_all[c * D : (c + 1) * D]
        # v1 [token, d+1] -> [p, j, d+1] with token = j*128 + p
        v1 = np.ascontiguousarray(
            v1_all[:, c, :].reshape(T // P, P, D + 1).transpose(1, 0, 2)
        )
        in_maps.append(
            {"qT": np.ascontiguousarray(qT), "kT": np.ascontiguousarray(kT), "v1": v1}
        )
    return in_maps


def _host_post(outs):
    """Divide by denominators and reassemble [T, NUM_HEADS*D] fp32."""
    full = np.empty((T, NUM_HEADS * D), dtype=np.float32)
    for c, arr in enumerate(outs):
        # arr: [HPC, B, P, KT, D+1] = [h, s, p, qb, d]
        o_un = arr[..., :D]
        den = arr[..., D : D + 1]
        on = o_un / den
        # token = s*1024 + qb*128 + p -> [s, qb, p, h, d]
        blk = on.transpose(1, 3, 2, 0, 4).reshape(T, HPC * D)
        full[:, c * HPC * D : (c + 1) * HPC * D] = blk
    return full


def kernel(q, k, v, k_cache, v_cache, slot_mapping, seq_slot_mapping, **kw):
    nc = _get_model()
    in_maps = _host_prep(q, k, v, k_cache, v_cache, slot_mapping, seq_slot_mapping)
    res = run_bass_kernel_spmd(nc, in_maps, core_ids=list(range(NCORES)))
    outs = [np.asarray(res.results[c]["o"], dtype=np.float32) for c in range(NCORES)]
    return _host_post(outs)


# revision 37
# speedup vs baseline: 1.2931x; 1.2931x over previous
"""Trainium2 Bass kernel for nn_Attention_58428735095559.

Paged-KV-cache GQA causal prefill attention:
  B=8 seqs x S=1024 tokens, 32 q-heads / 8 kv-heads, head_dim=128.
  reference: scatter k/v into a 16384-slot cache by slot_mapping, gather
  per-token KV by seq_slot_mapping, then causal GQA attention.

Sharding: tensor-parallel over heads across 8 cores. Core c owns kv-head c
and q-heads 4c..4c+3.

Host prep (not on the device critical path):
  - resolve scatter->gather exactly (last write wins) and gather K_eff/V_eff
  - pre-transpose q and K_eff into [d, token] layout, append a ones column
    to V_eff (fused softmax-denominator trick), cast everything to bf16

Device kernel per core (bf16 matmuls, fp32 PSUM):
  - QK: sc[kv, q] = kT_chunk.T @ qT  (contraction over d on partitions),
    block-causal skip, packed into [128, 1024] PSUM chunks
  - exp on ACT in large chunk instructions (scale folded in), out bf16 SBUF
  - diagonal causal mask as a multiplicative upper-tri mask on DVE (4x bf16)
  - PV: ot[q, d+1] = ep_chunk.T @ [V|1]  accumulated over kv tiles in PSUM;
    column d holds the softmax denominator for free
  - DVE copies ot -> SBUF staging; DMA unnormalized output + denominators
  - final divide + relayout on host
"""

import numpy as np

try:
    import concourse.bass as bass  # noqa: F401
except ImportError:  # fresh shells without the repo on PYTHONPATH
    import sys

    for p in ("/opt/trn_rl_repo", "/root/.axon_site/_ro/trn_rl_repo"):
        if p not in sys.path:
            sys.path.insert(0, p)

import concourse.bass as bass  # noqa: F401
import concourse.bacc as bacc
import concourse.mybir as mybir
import concourse.tile as tile
from concourse.bass_utils import run_bass_kernel_spmd

# problem constants (hardcoded; kernel.py must be self-contained)
B, S = 8, 1024
NUM_HEADS, HEAD_DIM, NUM_KV_HEADS = 32, 128, 8
T = B * S
NUM_SLOTS = 16384
SCALE = 1.0 / float(np.sqrt(HEAD_DIM))
NCORES = 8
HPC = NUM_HEADS // NCORES  # q heads per core = 4
D = HEAD_DIM
P = 128
KT = S // P  # kv tiles per seq = 8
CHUNK = 1536  # score columns per PSUM chunk (3 banks)
TOTCOL = sum(S - P * ki for ki in range(KT))  # 4608 block-causal score cols

F32 = mybir.dt.float32
BF16 = mybir.dt.bfloat16
AF = mybir.ActivationFunctionType
ALU = mybir.AluOpType

# global column offset where kv-tile ki's q-range begins
GS = [0] * KT
for _ki in range(1, KT):
    GS[_ki] = GS[_ki - 1] + (S - P * (_ki - 1))


def _qk_chunks():
    """Pack the block-causal (ki, q-range) score pieces into CHUNK-column
    PSUM chunks, splitting at 512 (PSUM bank) and CHUNK boundaries.
    Returns [ [(off_in_chunk, length, ki, qlo), ...] per chunk ]."""
    nchunk = (TOTCOL + CHUNK - 1) // CHUNK
    chunks = [[] for _ in range(nchunk)]
    g = 0
    for ki in range(KT):
        qcur = P * ki
        rem = S - qcur
        while rem > 0:
            ci, off = divmod(g, CHUNK)
            ln = min(rem, 512 - (off % 512), CHUNK - off)
            chunks[ci].append((off, ln, ki, qcur))
            g += ln
            qcur += ln
            rem -= ln
    assert g == TOTCOL
    return chunks


QK_CHUNKS = _qk_chunks()
NCHUNK = len(QK_CHUNKS)


def build_model():
    nc = bacc.Bacc("TRN2", target_bir_lowering=False, debug=False)

    qT_t = nc.dram_tensor("qT", [HPC, P, T], BF16, kind="ExternalInput")
    kT_t = nc.dram_tensor("kT", [P, T], BF16, kind="ExternalInput")
    v1_t = nc.dram_tensor("v1", [P, T // P, D + 1], BF16, kind="ExternalInput")
    o_t = nc.dram_tensor("o", [HPC, B, P, KT, D + 1], BF16, kind="ExternalOutput")

    with tile.TileContext(nc) as tc:
        with (
            tc.tile_pool(name="constp", bufs=1) as constp,
            tc.tile_pool(name="epp", bufs=2) as epp,
            tc.tile_pool(name="stgp", bufs=3) as stgp,
            tc.tile_pool(name="scp", bufs=2, space="PSUM") as scp,
            tc.tile_pool(name="otp", bufs=2, space="PSUM") as otp,
        ):
            # resident inputs, in consumption order (h outer, s inner).
            # The first ~3 seqs' slices go on the ACT HWDGE ring (idle at
            # t=0); the bulk is issued from GPSIMD (SWDGE, separate queue)
            # interleaved into the emission loop so transfers stream in
            # behind the compute. Outputs own the SP HWDGE ring.
            # All inputs stream through the ACT HWDGE ring (idle at t=0) in
            # strict consumption order — the ring is FIFO, so the first
            # iterations' slices complete before the bulk regardless of how
            # the scheduler orders the issue instructions. Outputs own the
            # SP ring (packet-granular round-robin would otherwise starve
            # small-descriptor transfers behind big ones).
            qts = constp.tile([P, HPC * T], BF16, tag="qts", name="qts")
            kts = constp.tile([P, T], BF16, tag="kts")
            v1 = constp.tile([P, T // P, D + 1], BF16, tag="v1")
            # all inputs on the ACT HWDGE ring in consumption order (ring is
            # FIFO; packet-granular round-robin across rings would starve
            # small descriptors behind big ones). Outputs own the SP ring.
            S3 = 3 * S
            nc.scalar.dma_start(kts[:, 0:S], kT_t.ap()[:, 0:S])
            nc.scalar.dma_start(qts[:, 0:S], qT_t.ap()[0, :, 0:S])
            nc.scalar.dma_start(kts[:, S:S3], kT_t.ap()[:, S:S3])
            nc.scalar.dma_start(qts[:, S:S3], qT_t.ap()[0, :, S:S3])
            nc.scalar.dma_start(v1[:, 0 : 3 * KT, :], v1_t.ap()[:, 0 : 3 * KT, :])
            nc.scalar.dma_start(kts[:, S3:T], kT_t.ap()[:, S3:T])
            nc.scalar.dma_start(qts[:, S3:T], qT_t.ap()[0, :, S3:T])
            nc.scalar.dma_start(v1[:, 3 * KT :, :], v1_t.ap()[:, 3 * KT :, :])
            for hh in range(1, HPC):
                nc.scalar.dma_start(
                    qts[:, hh * T : (hh + 1) * T], qT_t.ap()[hh, :, :]
                )

            def emit_qk_chunk(h, s, ep, ci, split=None):
                """QK matmuls + exp + diag masks for one score chunk.
                split: optional column to break the exp into two ACTIVATEs
                (used on the last iteration so trailing PV can start early)."""
                pieces = QK_CHUNKS[ci]
                base = sum(
                    sum(ln for _, ln, _, _ in QK_CHUNKS[j]) for j in range(ci)
                )
                clen = sum(ln for _, ln, _, _ in pieces)
                sc = scp.tile([P, CHUNK], F32, tag="sc", name="sc")
                for off, ln, ki, qlo in pieces:
                    q0 = h * T + s * S + qlo
                    nc.tensor.matmul(
                        sc[:, off : off + ln],
                        lhsT=kts[:, s * S + P * ki : s * S + P * (ki + 1)],
                        rhs=qts[:, q0 : q0 + ln],
                        start=True,
                        stop=True,
                    )
                for lo, hi in (
                    [(0, clen)] if split is None else [(0, split), (split, clen)]
                ):
                    nc.scalar.activation(
                        ep[:, base + lo : base + hi],
                        sc[:, lo:hi],
                        AF.Exp,
                        scale=SCALE,
                    )
                # diag-block causal masks inside this chunk (GPSIMD; in-place)
                for ki in range(KT):
                    if base <= GS[ki] < base + clen:
                        dsl = ep[:, GS[ki] : GS[ki] + P]
                        # keep where q >= kv: iota = f - p >= 0
                        nc.gpsimd.affine_select(
                            out=dsl,
                            in_=dsl,
                            compare_op=ALU.is_ge,
                            fill=0.0,
                            base=0,
                            channel_multiplier=-1,
                            pattern=[[1, P]],
                        )

            def emit_pv(h, s, ep, stage, qb_range):
                """PV accumulation + staging copies for a range of q-blocks."""
                for qb in qb_range:
                    ot = otp.tile([P, D + 1], F32, tag="ot", name="ot")
                    for ki in range(qb + 1):
                        col = GS[ki] + P * (qb - ki)
                        nc.tensor.matmul(
                            ot[:, :],
                            lhsT=ep[:, col : col + P],
                            rhs=v1[:, KT * s + ki, :],
                            start=(ki == 0),
                            stop=(ki == qb),
                        )
                    nc.vector.tensor_copy(stage[:, qb, :], ot[:, :])

            # software-pipelined emission, h outer so inputs stream in order:
            # [QK(i) c0, c1] [PV(i-1) qb0-4] [QK(i) c2] [PV(i-1) qb5-7].
            # c2's matmuls must sit EARLY in the PE stream: the exp's wait
            # semaphore counts all preceding PE completions, so PV matmuls
            # emitted before c2 would delay chunk-2's exp by their runtime.
            # The first PV batch still covers the PE bubble while chunk 0's
            # exp vacates c2's PSUM slot. The last iteration interleaves its
            # own PV behind each chunk so the pipeline drains promptly.
            NIT = B * HPC
            prev = None
            for it in range(NIT + 1):
                cur = None
                if it < NIT:
                    h, s = divmod(it, B)
                    ep = epp.tile([P, TOTCOL], BF16, tag="ep", name="ep")
                    stage = stgp.tile(
                        [P, KT, D + 1], BF16, tag="stage", name="stage"
                    )
                    cur = (h, s, ep, stage)
                    for ci in range(NCHUNK - 1):
                        emit_qk_chunk(h, s, ep, ci)
                if prev is not None:
                    ph, ps, pep, pstage = prev
                    emit_pv(ph, ps, pep, pstage, range(0, 5))
                if cur is not None:
                    h, s, ep, stage = cur
                    if it == NIT - 1:
                        # split the last exp at chunk offset 1024 (global col
                        # 4096): qb4-5 read cols <= 4096, so they unblock
                        # after the first sub-ACTIVATE; only qb6-7 trail the
                        # very last one.
                        emit_qk_chunk(h, s, ep, NCHUNK - 1, split=1024)
                        if prev is not None:
                            ph, ps, pep, pstage = prev
                            emit_pv(ph, ps, pep, pstage, range(5, KT))
                            nc.sync.dma_start(
                                o_t.ap()[ph, ps, :, :, :], pstage[:, :, :]
                            )
                            prev = None
                        # tail: drain PV in dependency order, streaming the
                        # output DMA per q-block pair as copies complete
                        for qb in range(0, KT, 2):
                            emit_pv(h, s, ep, stage, range(qb, qb + 2))
                            nc.sync.dma_start(
                                o_t.ap()[h, s, :, qb : qb + 2, :],
                                stage[:, qb : qb + 2, :],
                            )
                        cur = None
                    else:
                        emit_qk_chunk(h, s, ep, NCHUNK - 1)
                if prev is not None:
                    ph, ps, pep, pstage = prev
                    emit_pv(ph, ps, pep, pstage, range(5, KT))
                    nc.sync.dma_start(o_t.ap()[ph, ps, :, :, :], pstage[:, :, :])
                prev = cur

    nc.compile()
    return nc


_NC = None


def _get_model():
    global _NC
    if _NC is None:
        _NC = build_model()
    return _NC


def _host_prep(q, k, v, k_cache, v_cache, slot_mapping, seq_slot_mapping):
    """Resolve the cache scatter/gather on host and build per-core inputs."""
    import ml_dtypes

    bf16 = ml_dtypes.bfloat16
    q = np.asarray(q, dtype=np.float32)
    k = np.asarray(k, dtype=np.float32)
    v = np.asarray(v, dtype=np.float32)
    k_cache = np.asarray(k_cache, dtype=np.float32)
    v_cache = np.asarray(v_cache, dtype=np.float32)
    sm = np.asarray(slot_mapping, dtype=np.int64)
    ssm = np.asarray(seq_slot_mapping, dtype=np.int64)

    # exact scatter->gather resolution (last write wins, like jax .at[].set)
    last_writer = np.full(NUM_SLOTS, -1, dtype=np.int64)
    last_writer[sm] = np.arange(T, dtype=np.int64)
    lw = last_writer[ssm]
    hit = lw >= 0
    lw_safe = np.where(hit, lw, 0)
    keff = np.where(hit[:, None], k[lw_safe], k_cache[ssm])  # [T, DKV] f32
    veff = np.where(hit[:, None], v[lw_safe], v_cache[ssm])

    qT_all = np.ascontiguousarray(q.T.astype(bf16))  # [4096, T]
    kT_all = np.ascontiguousarray(keff.T.astype(bf16))  # [1024, T]
    v1_all = np.empty((T, NUM_KV_HEADS, D + 1), dtype=bf16)
    v1_all[:, :, :D] = veff.reshape(T, NUM_KV_HEADS, D).astype(bf16)
    v1_all[:, :, D] = np.float32(1.0)

    in_maps = []
    for c in range(NCORES):
        qT = qT_all[c * HPC * D : (c + 1) * HPC * D].reshape(HPC, P, T)
        kT = kT_all[c * D : (c + 1) * D]
        # v1 [token, d+1] -> [p, j, d+1] with token = j*128 + p
        v1 = np.ascontiguousarray(
            v1_all[:, c, :].reshape(T // P, P, D + 1).transpose(1, 0, 2)
        )
        in_maps.append(
            {"qT": np.ascontiguousarray(qT), "kT": np.ascontiguousarray(kT), "v1": v1}
        )
    return in_maps


def _host_post(outs):
    """Divide by denominators and reassemble [T, NUM_HEADS*D] fp32."""
    full = np.empty((T, NUM_HEADS * D), dtype=np.float32)
    for c, arr in enumerate(outs):
        # arr: [HPC, B, P, KT, D+1] = [h, s, p, qb, d]
        o_un = arr[..., :D]
        den = arr[..., D : D + 1]
        on = o_un / den
        # token = s*1024 + qb*128 + p -> [s, qb, p, h, d]
        blk = on.transpose(1, 3, 2, 0, 4).reshape(T, HPC * D)
        full[:, c * HPC * D : (c + 1) * HPC * D] = blk
    return full


def kernel(q, k, v, k_cache, v_cache, slot_mapping, seq_slot_mapping, **kw):
    nc = _get_model()
    in_maps = _host_prep(q, k, v, k_cache, v_cache, slot_mapping, seq_slot_mapping)
    res = run_bass_kernel_spmd(nc, in_maps, core_ids=list(range(NCORES)))
    outs = [np.asarray(res.results[c]["o"], dtype=np.float32) for c in range(NCORES)]
    return _host_post(outs)
